# revision 14
# baseline (speedup 1.0000x reference)
"""AttentionBlock (GroupNorm + single-head self-attention + residual) on 8 TRN2 cores.

Strategy: pure data-parallel over batch (16 items -> 2 per core), no collectives.
All six big matmuls per item (Q, K, V, S=K^T Q, PV, proj) run in fp8-e4m3 with
perf_mode=DoubleRow (2 contraction sub-tiles per pass -> 2x PE throughput).
Weights are pre-scaled by 64 on the host (w ~ N(0, 1/c) would underflow fp8's
normal range); the 1/64 is folded into the PSUM evictions.  x is shipped as
bf16 (GroupNorm stats + residual tolerate it at rel-err ~6e-3 << the 2e-2 gate).

Per item (c=512 channels, n=1024 positions, 32 groups of 16 channels):
  - GroupNorm: bn_stats per channel-tile IN DMA-ARRIVAL ORDER, group-combine
    via block-diagonal selector matmul, rsqrt on DVE (fast-inverse-sqrt + 2
    Newton steps so the scalar engine's activation tables never swap) -> hn fp8.
  - Q,K: DoubleRow matmuls, both evicted on ACT (scale 1/64 + bias) -- the
    QK stretch has no other ACT work, and this frees DVE for the *other*
    item's GroupNorm, which is emitted interleaved so hn(1) is ready the
    moment item 1's QKV matmuls come up.
  - V computed TRANSPOSED: matmul(lhsT=hn, rhs=wv) -> vT [n, c], DVE evict.
  - S^T = K^T Q -> [j, i] tiles; eviction on ACT: e = exp(S*scale - 3) fp8.
    The -3 shift keeps e <= ~120 < fp8e4's 240 ceiling (fp8 downcast is
    NONSAT: overflow would be Inf); softmax cancels the shift exactly.
  - Denominators WITHOUT any elementwise pass over e: 16 accumulating
    ones(=1/4)-matmuls reduce e over j into a [1, n] PSUM row; they are
    emitted one round BEHIND the S tiles inside the interleave loops so the
    exp evictions are always ready (no PE wait) and no serial D-phase is
    left at the end.  A K=1 matmul broadcasts the row to 128 partitions and
    reciprocal_approx_fast gives recip = 4/D.
  - out = V @ e.  Item 0 evicts the PV PSUM on the otherwise-idle ACT as a
    pure 1/256 cast (exact, and independent of the D chain -- softmax
    normalization commutes with the projection) and applies recip at the
    proj eviction: o = psum*recip + (x + bpp) with x+bpp precomputed on ACT.
    Item 1 normalizes at the PV eviction (DVE, recip carries a 4x ou
    pre-scale) so its proj eviction is a scale+bias; the four tail
    evictions alternate ACT/DVE so the final drain runs two-wide.
HAM control: the PE's clock gate re-throttles to 1.2 GHz after ~3.4us idle.
During the serial GroupNorm(0) window the PE runs small warmup bursts that
are *gated on GroupNorm progress* (each burst's operand is poked by a tiny
DVE copy), so the PE shows activity in every HAM window without ever
racing ahead of the real work; QKV then starts at full 2.4 GHz.
Startup: all small constants ride in ONE packed DMA; x tiles and fp8
weights are laid out across the sync/scalar/gpsimd trigger queues in the
order compute consumes them.  Outputs fan out over three DMA queues.
"""

import numpy as np
import ml_dtypes

B_TOT, C, H, W = 16, 512, 32, 32
N = H * W            # 1024
NCORES = 8
BPC = B_TOT // NCORES  # 2 batch items per core
CT = C // 128        # 4 channel tiles
NT = N // 128        # 8 position tiles
NCH = N // 512       # 2 free-dim chunks of 512
GS = 16              # group size (channels per group)
EPS = 1e-5
SCALE = float(C) ** -0.5
WS = 64.0            # weight pre-scale (folded out at evictions)
OUS = 4.0            # recip pre-scale (ones=1/4 -> recip_sb = 4/D)
OCAST = 1.0 / 256    # item0 PV eviction cast scale (pure shift, exact)
EXPB = -3.0          # exp logit shift (cancels in softmax)
NVEC = 5             # gamma, beta, bq, bk, bpp
CB_W = NVEC * CT + 128  # const blob width (vectors + sel)

_CACHE = {}


def _build_bass():
    import concourse.bass as bass  # noqa: F401
    import concourse.tile as tile
    from concourse import bacc, mybir

    F32 = mybir.dt.float32
    BF16 = mybir.dt.bfloat16
    F8 = mybir.dt.float8e4
    Alu = mybir.AluOpType
    Act = mybir.ActivationFunctionType
    DR = mybir.MatmulPerfMode.DoubleRow

    nc = bacc.Bacc("TRN2", target_bir_lowering=False, debug=False,
                   num_devices=NCORES)

    x_ext = nc.dram_tensor("x", [BPC, 128, CT, N], BF16, kind="ExternalInput").ap()
    w_ext = {
        name: nc.dram_tensor(name, [128, CT, 512], F8, kind="ExternalInput").ap()
        for name in ("wq", "wk", "wv", "wp")
    }
    cb_ext = nc.dram_tensor("cb", [128, CB_W], F32, kind="ExternalInput").ap()
    out_ext = nc.dram_tensor("out", [BPC, 128, CT, N], F32, kind="ExternalOutput").ap()

    with tile.TileContext(nc) as tc:
        with (
            tc.tile_pool(name="consts", bufs=1) as consts,
            tc.tile_pool(name="xp", bufs=2) as xp,
            tc.tile_pool(name="hnp", bufs=2) as hnp,
            tc.tile_pool(name="qkp", bufs=2) as qkp,
            tc.tile_pool(name="vp", bufs=2) as vp,
            tc.tile_pool(name="ep", bufs=2) as ep,
            tc.tile_pool(name="oup", bufs=2) as oup,
            tc.tile_pool(name="outp", bufs=3) as outp,
            tc.tile_pool(name="rp", bufs=2) as rp,
            tc.tile_pool(name="xbp", bufs=1) as xbp,
            tc.tile_pool(name="smallp", bufs=8) as smallp,
            tc.tile_pool(name="psq", bufs=2, space="PSUM") as psq,
            tc.tile_pool(name="psv", bufs=2, space="PSUM") as psv,
            tc.tile_pool(name="pssm", bufs=2, space="PSUM") as pssm,
        ):
            # ---- DMA plan (per-queue issue order == consumption order) ----
            # sync:   x0t0a x0t2 wk | x1t0 x1t1 x1t2    out: ot0, ot3
            # scalar: x0t0b x0t1 wq | x1 none           out: ot1
            # gpsimd: x0t3 cb wv wp | x1t3              out: ot2
            # x0-t0 ships as two half-tiles on separate queues: it gates the
            # very first bn_stats (subtile deps let [0:512] start early).
            def xt_tile(b, t):
                return xp.tile([128, N], BF16, tag=f"x{t}", name=f"x_b{b}_t{t}")

            x0 = [xt_tile(0, t) for t in range(CT)]
            x1 = [xt_tile(1, t) for t in range(CT)]
            w_sb = {
                name: consts.tile([128, CT, 512], F8, tag=name, name=f"w_{name}")
                for name in ("wq", "wk", "wv", "wp")
            }
            cb_sb = consts.tile([128, CB_W], F32, tag="cb")

            nc.sync.dma_start(x0[0][:, 0:512], x_ext[0, :, 0, 0:512])
            nc.scalar.dma_start(x0[0][:, 512:1024], x_ext[0, :, 0, 512:1024])
            nc.gpsimd.dma_start(x0[3][:], x_ext[0, :, 3, :])
            nc.sync.dma_start(x0[2][:], x_ext[0, :, 2, :])
            nc.scalar.dma_start(x0[1][:], x_ext[0, :, 1, :])
            nc.gpsimd.dma_start(cb_sb[:], cb_ext[:])
            nc.scalar.dma_start(w_sb["wq"][:], w_ext["wq"][:])
            nc.sync.dma_start(w_sb["wk"][:], w_ext["wk"][:])
            nc.gpsimd.dma_start(w_sb["wv"][:], w_ext["wv"][:])
            nc.sync.dma_start(x1[0][:], x_ext[1, :, 0, :])
            nc.gpsimd.dma_start(w_sb["wp"][:], w_ext["wp"][:])
            nc.sync.dma_start(x1[1][:], x_ext[1, :, 1, :])
            nc.gpsimd.dma_start(x1[3][:], x_ext[1, :, 3, :])
            nc.sync.dma_start(x1[2][:], x_ext[1, :, 2, :])

            vec_sb = {
                name: cb_sb[:, i * CT:(i + 1) * CT]
                for i, name in enumerate(("gamma", "beta", "bq", "bk", "bpp"))
            }
            sel_sb = cb_sb[:, NVEC * CT:NVEC * CT + 128]
            ones_sb = consts.tile([128, 1], BF16, tag="ones")
            nc.vector.memset(ones_sb[:], 1.0 / OUS)
            onescol_sb = consts.tile([1, 128], BF16, tag="onescol")
            nc.vector.memset(onescol_sb[:], 1.0)
            magic_sb = consts.tile([128, 1], mybir.dt.int32, tag="magic")
            nc.vector.memset(magic_sb[:], 0x5F3759DF)
            expb_sb = consts.tile([128, 1], F32, tag="expb")
            nc.vector.memset(expb_sb[:], EXPB)

            # ---- HAM-warming machinery ----
            wu_sb = consts.tile([128, 512], BF16, tag="wu")
            nc.vector.memset(wu_sb[:], 0.0)
            ps_wu = psv.tile([128, 512], F32, tag="vmm", name="ps_warm")
            wu_state = {"started": False}

            def warm_burst(k, stop=False):
                for i in range(k):
                    nc.tensor.matmul(ps_wu[:], wu_sb[:, 0:128], wu_sb[:],
                                     start=not wu_state["started"],
                                     stop=stop and i == k - 1)
                    wu_state["started"] = True

            def warm_poke(src):
                # tiny DVE write into the warmup operand: the next warm_burst
                # waits for it, so the PE's activity tracks GroupNorm progress
                # instead of racing ahead and then idling into a re-throttle.
                nc.vector.tensor_copy(wu_sb[:, 508:510], src)

            warm_burst(16)

            def gn_stats(b, xts, mv, t):
                stats = smallp.tile([128, 2, 6], F32, tag="stats",
                                    name=f"st{b}_{t}")
                nc.vector.bn_stats(stats[:, 0, :], xts[t][:, 0:512])
                nc.vector.bn_stats(stats[:, 1, :], xts[t][:, 512:1024])
                nc.vector.bn_aggr(mv[:, t, :], stats[:])

            def gn_tail(b, xts, mv, warm=False):
                # s_all[:, 0, t]=mean_t, s_all[:, 1, t]=E[x^2]_t
                s_all = smallp.tile([128, 2, CT], F32, tag="s_all", name=f"s{b}")
                nc.vector.tensor_copy(s_all[:, 0, :], mv[:, :, 0])
                nc.vector.tensor_tensor(s_all[:, 1, :], mv[:, :, 0], mv[:, :, 0],
                                        Alu.mult)
                nc.vector.tensor_tensor(s_all[:, 1, :], s_all[:, 1, :],
                                        mv[:, :, 1], Alu.add)
                gs = pssm.tile([128, 2, CT], F32, tag="sm", name=f"gs{b}")
                nc.tensor.matmul(gs[:], sel_sb, s_all[:], start=True, stop=True)
                gsb = smallp.tile([128, 2, CT], F32, tag="gsb", name=f"gb{b}")
                nc.vector.tensor_copy(gsb[:], gs[:])
                ab = smallp.tile([128, 4, CT], F32, tag="ab", name=f"ab{b}")
                va = ab[:, 0, :]         # var
                vp_ = ab[:, 1, :]        # var + eps
                y = ab[:, 2, :]
                tmp = ab[:, 3, :]
                nc.vector.tensor_tensor(va, gsb[:, 0, :], gsb[:, 0, :], Alu.mult)
                nc.vector.tensor_tensor(va, gsb[:, 1, :], va, Alu.subtract)
                # rstd = rsqrt(var+eps) entirely on DVE (fast-inverse-sqrt seed
                # + 2 Newton steps) so the scalar engine's activation tables
                # never leave the exp set (table reloads are 2.7us each).
                nc.vector.tensor_scalar_add(vp_, va, EPS)
                I32 = mybir.dt.int32
                nc.vector.tensor_scalar(y.bitcast(I32), vp_.bitcast(I32), 1,
                                        None, Alu.arith_shift_right)
                nc.vector.tensor_tensor(y.bitcast(I32),
                                        magic_sb[:].to_broadcast([128, CT]),
                                        y.bitcast(I32), Alu.subtract)
                for _ in range(2):  # Newton: y *= 1.5 - 0.5*v*y^2
                    nc.vector.tensor_tensor(tmp, y, y, Alu.mult)
                    nc.vector.tensor_tensor(tmp, tmp, vp_, Alu.mult)
                    nc.vector.tensor_scalar(tmp, tmp, -0.5, 1.5, Alu.mult,
                                            Alu.add)
                    nc.vector.tensor_tensor(y, y, tmp, Alu.mult)
                if warm:
                    warm_poke(y[:, 0:2])
                    warm_burst(6)
                a_all = ab[:, 0, :]      # reuse var slot: a = rstd*gamma
                bsh = ab[:, 3, :]
                nc.vector.tensor_tensor(a_all, y, vec_sb["gamma"], Alu.mult)
                nc.vector.tensor_tensor(bsh, gsb[:, 0, :], a_all, Alu.mult)
                nc.vector.tensor_tensor(bsh, vec_sb["beta"], bsh, Alu.subtract)
                hn_sb = hnp.tile([128, CT, N], F8, tag="hn", name=f"hn{b}")
                for t in range(CT):
                    if t < 2:
                        nc.vector.tensor_scalar(hn_sb[:, t, :], xts[t][:],
                                                ab[:, 0, t:t + 1],
                                                ab[:, 3, t:t + 1],
                                                Alu.mult, Alu.add)
                    else:
                        nc.scalar.activation(hn_sb[:, t, :], xts[t][:],
                                             Act.Identity,
                                             bias=ab[:, 3, t:t + 1],
                                             scale=ab[:, 0, t:t + 1])
                    if warm and t == 1:
                        warm_poke(ab[:, 0, 0:2])
                        warm_burst(6)
                    if warm and t == 2:
                        # poke reads the ACT-evicted hn tile: the final burst
                        # only starts once t2 is applied, covering t3's window
                        nc.vector.tensor_copy(wu_sb[:, 506:508],
                                              hn_sb[:, 2, 0:2])
                        warm_burst(6, stop=True)
                return hn_sb

            def qk_tile(b, hn_sb, dst, wname, bname, t, on_act):
                # dst[:, t, :] = psum/WS + bias, psum = w^T @ hn (DoubleRow)
                ps = psq.tile([128, N], F32, tag="mm", name=f"ps_{wname}{b}_{t}")
                for itp in range(2):
                    lhs = w_sb[wname][:, 2 * itp:2 * itp + 2, t * 128:(t + 1) * 128]
                    for ch in range(NCH):
                        cs = slice(ch * 512, (ch + 1) * 512)
                        nc.tensor.matmul(ps[:, cs], lhs,
                                         hn_sb[:, 2 * itp:2 * itp + 2, cs],
                                         start=(itp == 0), stop=(itp == 1),
                                         perf_mode=DR)
                bias = vec_sb[bname][:, t:t + 1]
                if on_act:
                    nc.scalar.activation(dst[:, t, :], ps[:], Act.Identity,
                                         bias=bias, scale=1.0 / WS)
                else:
                    nc.vector.tensor_scalar(dst[:, t, :], ps[:], 1.0 / WS,
                                            bias, Alu.mult, Alu.add)

            def v_tile(b, hn_sb, vT_sb, jt):
                # vT[:, jt, :] = (hn^T @ wv)/WS  (DoubleRow, transposed out)
                ps = psv.tile([128, 512], F32, tag="vmm", name=f"psv{b}_{jt}")
                for itp in range(2):
                    nc.tensor.matmul(
                        ps[:], hn_sb[:, 2 * itp:2 * itp + 2, jt * 128:(jt + 1) * 128],
                        w_sb["wv"][:, 2 * itp:2 * itp + 2, :],
                        start=(itp == 0), stop=(itp == 1), perf_mode=DR)
                nc.vector.tensor_scalar(vT_sb[:, jt, :], ps[:], 1.0 / WS,
                                        None, Alu.mult)

            def s_tile(b, q_sb, k_sb, e_sb, jt):
                # e[:, jt, :] = exp(scale * k[:, :, jt-tile]^T @ q + EXPB)
                ps = psq.tile([128, N], F32, tag="mm", name=f"pss{b}_{jt}")
                for ctp in range(2):
                    lhs = k_sb[:, 2 * ctp:2 * ctp + 2, jt * 128:(jt + 1) * 128]
                    for ch in range(NCH):
                        cs = slice(ch * 512, (ch + 1) * 512)
                        nc.tensor.matmul(ps[:, cs], lhs,
                                         q_sb[:, 2 * ctp:2 * ctp + 2, cs],
                                         start=(ctp == 0), stop=(ctp == 1),
                                         perf_mode=DR)
                nc.scalar.activation(e_sb[:, jt, :], ps[:], Act.Exp,
                                     bias=expb_sb[:], scale=SCALE)

            def dsum_make(b):
                psd = [pssm.tile([1, 512], F32, tag="sm", name=f"d{b}_{ch}")
                       for ch in range(NCH)]
                return psd

            def dsum_acc(b, psd, e_sb, jts):
                # accumulate D rows for the given j-tiles (emitted one round
                # behind the S tiles so the exp evictions are always ready)
                for ch in range(NCH):
                    cs = slice(ch * 512, (ch + 1) * 512)
                    for jt in jts:
                        nc.tensor.matmul(psd[ch][:], ones_sb[:],
                                         e_sb[:, jt, cs],
                                         start=(jt == 0), stop=(jt == NT - 1))

            def dsum_tail(b, psd):
                drow = rp.tile([1, N], BF16, tag="drow", name=f"dr{b}")
                recip_sb = rp.tile([128, N], F32, tag="recip", name=f"rc{b}")
                for ch in range(NCH):
                    cs = slice(ch * 512, (ch + 1) * 512)
                    nc.scalar.copy(drow[:, cs], psd[ch][:])
                for ch in range(NCH):
                    cs = slice(ch * 512, (ch + 1) * 512)
                    bc = pssm.tile([128, 512], F32, tag="sm", name=f"bc{b}_{ch}")
                    nc.tensor.matmul(bc[:], onescol_sb[:], drow[:, cs],
                                     start=True, stop=True)
                    nc.vector.reciprocal_approx_fast(recip_sb[:, cs], bc[:])
                return recip_sb

            def pv_mms(b, vT_sb, e_sb, ct):
                ps = psq.tile([128, N], F32, tag="mm", name=f"pso{b}_{ct}")
                for jtp in range(4):
                    lhs = vT_sb[:, 2 * jtp:2 * jtp + 2, ct * 128:(ct + 1) * 128]
                    for ch in range(NCH):
                        cs = slice(ch * 512, (ch + 1) * 512)
                        nc.tensor.matmul(ps[:, cs], lhs,
                                         e_sb[:, 2 * jtp:2 * jtp + 2, cs],
                                         start=(jtp == 0), stop=(jtp == 3),
                                         perf_mode=DR)
                return ps

            out_engs = [nc.sync, nc.scalar, nc.gpsimd, nc.sync]

            def proj_tile(b, ou_sb, ot, evict):
                # evict: ('recip', recip_sb, xb_sb) -> o = ps*recip + xb
                #        ('act'|'dve', xts)        -> o = ps/(WS*OUS)+bpp, + x
                ps = psq.tile([128, N], F32, tag="mm", name=f"psp{b}_{ot}")
                for ctp in range(2):
                    lhs = w_sb["wp"][:, 2 * ctp:2 * ctp + 2, ot * 128:(ot + 1) * 128]
                    for ch in range(NCH):
                        cs = slice(ch * 512, (ch + 1) * 512)
                        nc.tensor.matmul(ps[:, cs], lhs,
                                         ou_sb[:, 2 * ctp:2 * ctp + 2, cs],
                                         start=(ctp == 0), stop=(ctp == 1),
                                         perf_mode=DR)
                o_sb = outp.tile([128, N], F32, tag="o", name=f"o{b}_{ot}")
                bias = vec_sb["bpp"][:, ot:ot + 1]
                if evict[0] == 'recip':
                    _, recip_sb, xb_sb = evict
                    nc.vector.tensor_tensor(o_sb[:], ps[:], recip_sb[:],
                                            Alu.mult)
                    nc.vector.tensor_tensor(o_sb[:], o_sb[:], xb_sb[:, ot, :],
                                            Alu.add)
                else:
                    kind, xts = evict
                    if kind == 'act':
                        nc.scalar.activation(o_sb[:], ps[:], Act.Identity,
                                             bias=bias, scale=1.0 / (WS * OUS))
                    else:
                        nc.vector.tensor_scalar(o_sb[:], ps[:],
                                                1.0 / (WS * OUS), bias,
                                                Alu.mult, Alu.add)
                    nc.vector.tensor_tensor(o_sb[:], o_sb[:], xts[ot][:],
                                            Alu.add)
                out_engs[ot].dma_start(out_ext[b, :, ot, :], o_sb[:])

            # ---- software pipeline over the two batch items ----
            # GroupNorm(0) with warmup bursts gated on its progress;
            # stats run in DMA-arrival order (t3 lands first on gpsimd).
            mv0 = smallp.tile([128, CT, 2], F32, tag="mv", name="mv0")
            for t in (0, 3, 2, 1):
                gn_stats(0, x0, mv0, t)
                warm_poke(mv0[:, t, :])
                warm_burst(6)
            h0 = gn_tail(0, x0, mv0, warm=True)

            # Q/K stretch for item 0 (ACT evictions) with item 1's GroupNorm
            # stats interleaved on the otherwise-idle DVE.
            q0 = qkp.tile([128, CT, N], F8, tag="q", name="q0")
            k0 = qkp.tile([128, CT, N], F8, tag="k", name="k0")
            mv1 = smallp.tile([128, CT, 2], F32, tag="mv", name="mv1")
            for t in range(CT):
                qk_tile(0, h0, k0, "wk", "bk", t, on_act=True)
                qk_tile(0, h0, q0, "wq", "bq", t, on_act=False)
                gn_stats(1, x1, mv1, t)
            h1 = gn_tail(1, x1, mv1)

            v0 = vp.tile([128, NT, 512], F8, tag="vT", name="vT0")
            for jt in range(NT):
                v_tile(0, h0, v0, jt)

            # item0 S-phase interleaved with item1 QKV + item0 D-sums (lagged)
            e0 = ep.tile([128, NT, N], F8, tag="e", name="e0")
            q1 = qkp.tile([128, CT, N], F8, tag="q", name="q1")
            k1 = qkp.tile([128, CT, N], F8, tag="k", name="k1")
            v1 = vp.tile([128, NT, 512], F8, tag="vT", name="vT1")
            psd0 = dsum_make(0)
            for r in range(CT):
                s_tile(0, q0, k0, e0, 2 * r)
                s_tile(0, q0, k0, e0, 2 * r + 1)
                qk_tile(1, h1, k1, "wk", "bk", r, on_act=True)
                qk_tile(1, h1, q1, "wq", "bq", r, on_act=False)
                v_tile(1, h1, v1, 2 * r)
                v_tile(1, h1, v1, 2 * r + 1)
                if r > 0:
                    dsum_acc(0, psd0, e0, (2 * r - 2, 2 * r - 1))
            dsum_acc(0, psd0, e0, (NT - 2, NT - 1))
            # pv0: PSUM evicted on ACT as a pure 1/256 cast (no recip
            # dependency); normalization happens at proj0's eviction.
            ou0 = oup.tile([128, CT, N], F8, tag="ou", name="ou0")
            for ct in range(CT):
                ps = pv_mms(0, v0, e0, ct)
                nc.scalar.mul(ou0[:, ct, :], ps[:], OCAST)
            r0 = dsum_tail(0, psd0)
            xb0 = xbp.tile([128, CT, N], BF16, tag="xb", name="xb0")
            for t in range(CT):
                nc.scalar.activation(xb0[:, t, :], x0[t][:], Act.Identity,
                                     bias=vec_sb["bpp"][:, t:t + 1])

            # item0 proj interleaved with item1 S-phase + item1 D-sums
            e1 = ep.tile([128, NT, N], F8, tag="e", name="e1")
            psd1 = dsum_make(1)
            for r in range(CT):
                proj_tile(0, ou0, r, ('recip', r0, xb0))
                s_tile(1, q1, k1, e1, 2 * r)
                s_tile(1, q1, k1, e1, 2 * r + 1)
                if r > 0:
                    dsum_acc(1, psd1, e1, (2 * r - 2, 2 * r - 1))
            # pv1-ct0's matmuls cover the wait for the last exp1 eviction
            ou1 = oup.tile([128, CT, N], F8, tag="ou", name="ou1")
            dsum_acc(1, psd1, e1, (NT - 2,))
            ps10 = pv_mms(1, v1, e1, 0)
            dsum_acc(1, psd1, e1, (NT - 1,))
            r1 = dsum_tail(1, psd1)
            nc.vector.tensor_tensor(ou1[:, 0, :], ps10[:], r1[:], Alu.mult)
            for ct in range(1, CT):
                ps = pv_mms(1, v1, e1, ct)
                nc.vector.tensor_tensor(ou1[:, ct, :], ps[:], r1[:], Alu.mult)
            for r in range(CT):
                proj_tile(1, ou1, r, ('act' if r % 2 == 0 else 'dve', x1))

    nc.compile()
    return nc


def _prep_vec(v):
    # [C] f32 -> [128, CT] with v_sb[p, t] = v[t*128 + p]
    return np.ascontiguousarray(
        np.asarray(v, dtype=np.float32).reshape(CT, 128).T)


def _prep_w(w):
    # [C, C] (out, in) -> lhsT layout [128, CT, 512] fp8e4, pre-scaled by WS:
    # w_sb[p, it, o] = w[o, it*128 + p] * WS
    wT = np.asarray(w, dtype=np.float32).T * WS
    arr = wT.reshape(CT, 128, C).transpose(1, 0, 2)
    return np.clip(np.ascontiguousarray(arr), -240.0, 240.0).astype(
        ml_dtypes.float8_e4m3)


def kernel(x, gamma, beta, wq, bq, wk, bk, wv, bv, wp, bp):
    from concourse.bass_utils import run_bass_kernel_spmd

    nc = _CACHE.get("nc")
    if nc is None:
        nc = _CACHE["nc"] = _build_bass()

    x = np.asarray(x, dtype=np.float32)
    # [16, C, H, W] -> [16, 128, CT, N] bf16
    xr = np.ascontiguousarray(
        x.reshape(B_TOT, CT, 128, N).transpose(0, 2, 1, 3)).astype(
        ml_dtypes.bfloat16)

    bpp = np.asarray(wp, np.float32) @ np.asarray(bv, np.float32) \
        + np.asarray(bp, np.float32)
    sel = np.kron(np.eye(128 // GS, dtype=np.float32),
                  np.full((GS, GS), 1.0 / GS, dtype=np.float32))
    cb = np.empty((128, CB_W), dtype=np.float32)
    for i, v in enumerate((gamma, beta, bq, bk, bpp)):
        cb[:, i * CT:(i + 1) * CT] = _prep_vec(v)
    cb[:, NVEC * CT:] = sel
    common = {
        "wq": _prep_w(wq), "wk": _prep_w(wk), "wv": _prep_w(wv),
        "wp": _prep_w(wp), "cb": cb,
    }
    in_maps = [
        {"x": np.ascontiguousarray(xr[c * BPC:(c + 1) * BPC]), **common}
        for c in range(NCORES)
    ]
    res = run_bass_kernel_spmd(nc, in_maps, core_ids=list(range(NCORES)))
    # [BPC, 128, CT, N] per core -> [16, C, H, W]
    out = np.concatenate([r["out"] for r in res.results], axis=0)
    return np.ascontiguousarray(
        out.transpose(0, 2, 1, 3)).reshape(B_TOT, C, H, W)


# revision 15
# speedup vs baseline: 1.0195x; 1.0195x over previous
"""AttentionBlock (GroupNorm + single-head self-attention + residual) on 8 TRN2 cores.

Strategy: pure data-parallel over batch (16 items -> 2 per core), no collectives.
All six big matmuls per item (Q, K, V, S=K^T Q, PV, proj) run in fp8-e4m3 with
perf_mode=DoubleRow (2 contraction sub-tiles per pass -> 2x PE throughput).
Weights are pre-scaled by 64 on the host (w ~ N(0, 1/c) would underflow fp8's
normal range); the 1/64 is folded into the PSUM evictions.  x is shipped as
bf16 (GroupNorm stats + residual tolerate it at rel-err ~6e-3 << the 2e-2 gate).

Per item (c=512 channels, n=1024 positions, 32 groups of 16 channels):
  - GroupNorm: bn_stats per channel-tile IN DMA-ARRIVAL ORDER, group-combine
    via block-diagonal selector matmul, rsqrt on DVE (fast-inverse-sqrt + 2
    Newton steps so the scalar engine's activation tables never swap) -> hn fp8.
  - Q,K: DoubleRow matmuls, both evicted on ACT (scale 1/64 + bias) -- the
    QK stretch has no other ACT work, and this frees DVE for the *other*
    item's GroupNorm, which is emitted interleaved so hn(1) is ready the
    moment item 1's QKV matmuls come up.
  - V computed TRANSPOSED: matmul(lhsT=hn, rhs=wv) -> vT [n, c], DVE evict.
  - S^T = K^T Q -> [j, i] tiles; eviction on ACT: e = exp(S*scale - 3) fp8.
    The -3 shift keeps e <= ~120 < fp8e4's 240 ceiling (fp8 downcast is
    NONSAT: overflow would be Inf); softmax cancels the shift exactly.
  - Denominators WITHOUT any elementwise pass over e: 16 accumulating
    ones(=1/4)-matmuls reduce e over j into a [1, n] PSUM row; they are
    emitted one round BEHIND the S tiles inside the interleave loops so the
    exp evictions are always ready (no PE wait) and no serial D-phase is
    left at the end.  A K=1 matmul broadcasts the row to 128 partitions and
    reciprocal_approx_fast gives recip = 4/D.
  - out = V @ e.  Item 0 evicts the PV PSUM on the otherwise-idle ACT as a
    pure 1/256 cast (exact, and independent of the D chain -- softmax
    normalization commutes with the projection) and applies recip at the
    proj eviction: o = psum*recip + (x + bpp) with x+bpp precomputed on ACT.
    Item 1 normalizes at the PV eviction (DVE, recip carries a 4x ou
    pre-scale) so its proj eviction is a scale+bias; the four tail
    evictions alternate ACT/DVE so the final drain runs two-wide.
HAM control: the PE's clock gate re-throttles to 1.2 GHz after ~3.4us idle.
During the serial GroupNorm(0) window the PE runs small warmup bursts that
are *gated on GroupNorm progress* (each burst's operand is poked by a tiny
DVE copy), so the PE shows activity in every HAM window without ever
racing ahead of the real work; QKV then starts at full 2.4 GHz.
Startup: all small constants ride in ONE packed DMA; x tiles and fp8
weights are laid out across the sync/scalar/gpsimd trigger queues in the
order compute consumes them.  Outputs fan out over three DMA queues.
"""

import numpy as np
import ml_dtypes

B_TOT, C, H, W = 16, 512, 32, 32
N = H * W            # 1024
NCORES = 8
BPC = B_TOT // NCORES  # 2 batch items per core
CT = C // 128        # 4 channel tiles
NT = N // 128        # 8 position tiles
NCH = N // 512       # 2 free-dim chunks of 512
GS = 16              # group size (channels per group)
EPS = 1e-5
SCALE = float(C) ** -0.5
WS = 64.0            # weight pre-scale (folded out at evictions)
OUS = 4.0            # recip pre-scale (ones=1/4 -> recip_sb = 4/D)
OCAST = 1.0 / 256    # item0 PV eviction cast scale (pure shift, exact)
EXPB = -3.0          # exp logit shift (cancels in softmax)
NVEC = 5             # gamma, beta, bq, bk, bpp
CB_W = NVEC * CT + 128  # const blob width (vectors + sel)

_CACHE = {}


def _build_bass():
    import concourse.bass as bass  # noqa: F401
    import concourse.tile as tile
    from concourse import bacc, mybir

    F32 = mybir.dt.float32
    BF16 = mybir.dt.bfloat16
    F8 = mybir.dt.float8e4
    Alu = mybir.AluOpType
    Act = mybir.ActivationFunctionType
    DR = mybir.MatmulPerfMode.DoubleRow

    nc = bacc.Bacc("TRN2", target_bir_lowering=False, debug=False,
                   num_devices=NCORES)

    x_ext = nc.dram_tensor("x", [BPC, 128, CT, N], BF16, kind="ExternalInput").ap()
    w_ext = {
        name: nc.dram_tensor(name, [128, CT, 512], F8, kind="ExternalInput").ap()
        for name in ("wq", "wk", "wv", "wp")
    }
    cb_ext = nc.dram_tensor("cb", [128, CB_W], F32, kind="ExternalInput").ap()
    out_ext = nc.dram_tensor("out", [BPC, 128, CT, N], F32, kind="ExternalOutput").ap()

    with tile.TileContext(nc) as tc:
        with (
            tc.tile_pool(name="consts", bufs=1) as consts,
            tc.tile_pool(name="xp", bufs=2) as xp,
            tc.tile_pool(name="hnp", bufs=2) as hnp,
            tc.tile_pool(name="qkp", bufs=2) as qkp,
            tc.tile_pool(name="vp", bufs=2) as vp,
            tc.tile_pool(name="ep", bufs=2) as ep,
            tc.tile_pool(name="oup", bufs=2) as oup,
            tc.tile_pool(name="outp", bufs=3) as outp,
            tc.tile_pool(name="rp", bufs=2) as rp,
            tc.tile_pool(name="xbp", bufs=1) as xbp,
            tc.tile_pool(name="smallp", bufs=8) as smallp,
            tc.tile_pool(name="psq", bufs=2, space="PSUM") as psq,
            tc.tile_pool(name="psv", bufs=2, space="PSUM") as psv,
            tc.tile_pool(name="pssm", bufs=2, space="PSUM") as pssm,
        ):
            # ---- DMA plan (per-queue issue order == consumption order) ----
            # sync:   x0t0a x0t2 wk | x1t0 x1t1 x1t2    out: ot0, ot3
            # scalar: x0t0b x0t1 wq | x1 none           out: ot1
            # gpsimd: x0t3 cb wv wp | x1t3              out: ot2
            # x0-t0 ships as two half-tiles on separate queues: it gates the
            # very first bn_stats (subtile deps let [0:512] start early).
            def xt_tile(b, t):
                return xp.tile([128, N], BF16, tag=f"x{t}", name=f"x_b{b}_t{t}")

            x0 = [xt_tile(0, t) for t in range(CT)]
            x1 = [xt_tile(1, t) for t in range(CT)]
            w_sb = {
                name: consts.tile([128, CT, 512], F8, tag=name, name=f"w_{name}")
                for name in ("wq", "wk", "wv", "wp")
            }
            cb_sb = consts.tile([128, CB_W], F32, tag="cb")

            nc.sync.dma_start(x0[0][:, 0:512], x_ext[0, :, 0, 0:512])
            nc.scalar.dma_start(x0[0][:, 512:1024], x_ext[0, :, 0, 512:1024])
            nc.gpsimd.dma_start(x0[3][:], x_ext[0, :, 3, :])
            nc.sync.dma_start(x0[2][:], x_ext[0, :, 2, :])
            nc.scalar.dma_start(x0[1][:], x_ext[0, :, 1, :])
            nc.gpsimd.dma_start(cb_sb[:], cb_ext[:])
            nc.scalar.dma_start(w_sb["wq"][:], w_ext["wq"][:])
            nc.sync.dma_start(w_sb["wk"][:], w_ext["wk"][:])
            nc.gpsimd.dma_start(w_sb["wv"][:], w_ext["wv"][:])
            nc.sync.dma_start(x1[0][:], x_ext[1, :, 0, :])
            nc.gpsimd.dma_start(w_sb["wp"][:], w_ext["wp"][:])
            nc.sync.dma_start(x1[1][:], x_ext[1, :, 1, :])
            nc.gpsimd.dma_start(x1[3][:], x_ext[1, :, 3, :])
            nc.sync.dma_start(x1[2][:], x_ext[1, :, 2, :])

            vec_sb = {
                name: cb_sb[:, i * CT:(i + 1) * CT]
                for i, name in enumerate(("gamma", "beta", "bq", "bk", "bpp"))
            }
            sel_sb = cb_sb[:, NVEC * CT:NVEC * CT + 128]
            ones_sb = consts.tile([128, 1], BF16, tag="ones")
            nc.vector.memset(ones_sb[:], 1.0 / OUS)
            onescol_sb = consts.tile([1, 128], BF16, tag="onescol")
            nc.vector.memset(onescol_sb[:], 1.0)
            magic_sb = consts.tile([128, 1], mybir.dt.int32, tag="magic")
            nc.vector.memset(magic_sb[:], 0x5F3759DF)
            expb_sb = consts.tile([128, 1], F32, tag="expb")
            nc.vector.memset(expb_sb[:], EXPB)

            # ---- HAM-warming machinery ----
            wu_sb = consts.tile([128, 512], BF16, tag="wu")
            nc.vector.memset(wu_sb[:], 0.0)
            ps_wu = psv.tile([128, 512], F32, tag="vmm", name="ps_warm")
            wu_state = {"started": False}

            def warm_burst(k, stop=False):
                for i in range(k):
                    nc.tensor.matmul(ps_wu[:], wu_sb[:, 0:128], wu_sb[:],
                                     start=not wu_state["started"],
                                     stop=stop and i == k - 1)
                    wu_state["started"] = True

            def warm_poke(src):
                # tiny DVE write into the warmup operand: the next warm_burst
                # waits for it, so the PE's activity tracks GroupNorm progress
                # instead of racing ahead and then idling into a re-throttle.
                nc.vector.tensor_copy(wu_sb[:, 508:510], src)

            warm_burst(16)

            def gn_stats(b, xts, mv, t):
                stats = smallp.tile([128, 2, 6], F32, tag="stats",
                                    name=f"st{b}_{t}")
                nc.vector.bn_stats(stats[:, 0, :], xts[t][:, 0:512])
                nc.vector.bn_stats(stats[:, 1, :], xts[t][:, 512:1024])
                nc.vector.bn_aggr(mv[:, t, :], stats[:])

            def gn_tail(b, xts, mv, warm=False):
                # s_all[:, 0, t]=mean_t, s_all[:, 1, t]=E[x^2]_t
                s_all = smallp.tile([128, 2, CT], F32, tag="s_all", name=f"s{b}")
                nc.vector.tensor_copy(s_all[:, 0, :], mv[:, :, 0])
                nc.vector.tensor_tensor(s_all[:, 1, :], mv[:, :, 0], mv[:, :, 0],
                                        Alu.mult)
                nc.vector.tensor_tensor(s_all[:, 1, :], s_all[:, 1, :],
                                        mv[:, :, 1], Alu.add)
                gs = pssm.tile([128, 2, CT], F32, tag="sm", name=f"gs{b}")
                nc.tensor.matmul(gs[:], sel_sb, s_all[:], start=True, stop=True)
                gsb = smallp.tile([128, 2, CT], F32, tag="gsb", name=f"gb{b}")
                nc.vector.tensor_copy(gsb[:], gs[:])
                ab = smallp.tile([128, 4, CT], F32, tag="ab", name=f"ab{b}")
                va = ab[:, 0, :]         # var
                vp_ = ab[:, 1, :]        # var + eps
                y = ab[:, 2, :]
                tmp = ab[:, 3, :]
                nc.vector.tensor_tensor(va, gsb[:, 0, :], gsb[:, 0, :], Alu.mult)
                nc.vector.tensor_tensor(va, gsb[:, 1, :], va, Alu.subtract)
                # rstd = rsqrt(var+eps) entirely on DVE (fast-inverse-sqrt seed
                # + 2 Newton steps) so the scalar engine's activation tables
                # never leave the exp set (table reloads are 2.7us each).
                nc.vector.tensor_scalar_add(vp_, va, EPS)
                I32 = mybir.dt.int32
                nc.vector.tensor_scalar(y.bitcast(I32), vp_.bitcast(I32), 1,
                                        None, Alu.arith_shift_right)
                nc.vector.tensor_tensor(y.bitcast(I32),
                                        magic_sb[:].to_broadcast([128, CT]),
                                        y.bitcast(I32), Alu.subtract)
                for _ in range(2):  # Newton: y *= 1.5 - 0.5*v*y^2
                    nc.vector.tensor_tensor(tmp, y, y, Alu.mult)
                    nc.vector.tensor_tensor(tmp, tmp, vp_, Alu.mult)
                    nc.vector.tensor_scalar(tmp, tmp, -0.5, 1.5, Alu.mult,
                                            Alu.add)
                    nc.vector.tensor_tensor(y, y, tmp, Alu.mult)
                if warm:
                    warm_poke(y[:, 0:2])
                    warm_burst(6)
                a_all = ab[:, 0, :]      # reuse var slot: a = rstd*gamma
                bsh = ab[:, 3, :]
                nc.vector.tensor_tensor(a_all, y, vec_sb["gamma"], Alu.mult)
                nc.vector.tensor_tensor(bsh, gsb[:, 0, :], a_all, Alu.mult)
                nc.vector.tensor_tensor(bsh, vec_sb["beta"], bsh, Alu.subtract)
                hn_sb = hnp.tile([128, CT, N], F8, tag="hn", name=f"hn{b}")
                for t in range(CT):
                    if t < 2:
                        nc.vector.tensor_scalar(hn_sb[:, t, :], xts[t][:],
                                                ab[:, 0, t:t + 1],
                                                ab[:, 3, t:t + 1],
                                                Alu.mult, Alu.add)
                    else:
                        nc.scalar.activation(hn_sb[:, t, :], xts[t][:],
                                             Act.Identity,
                                             bias=ab[:, 3, t:t + 1],
                                             scale=ab[:, 0, t:t + 1])
                    if warm and t == 1:
                        warm_poke(ab[:, 0, 0:2])
                        warm_burst(8, stop=True)
                return hn_sb

            def qk_tile(b, hn_sb, dst, wname, bname, t, on_act):
                # dst[:, t, :] = psum/WS + bias, psum = w^T @ hn (DoubleRow)
                ps = psq.tile([128, N], F32, tag="mm", name=f"ps_{wname}{b}_{t}")
                for itp in range(2):
                    lhs = w_sb[wname][:, 2 * itp:2 * itp + 2, t * 128:(t + 1) * 128]
                    for ch in range(NCH):
                        cs = slice(ch * 512, (ch + 1) * 512)
                        nc.tensor.matmul(ps[:, cs], lhs,
                                         hn_sb[:, 2 * itp:2 * itp + 2, cs],
                                         start=(itp == 0), stop=(itp == 1),
                                         perf_mode=DR)
                bias = vec_sb[bname][:, t:t + 1]
                if on_act:
                    nc.scalar.activation(dst[:, t, :], ps[:], Act.Identity,
                                         bias=bias, scale=1.0 / WS)
                else:
                    nc.vector.tensor_scalar(dst[:, t, :], ps[:], 1.0 / WS,
                                            bias, Alu.mult, Alu.add)

            def v_tile(b, hn_sb, vT_sb, jt):
                # vT[:, jt, :] = (hn^T @ wv)/WS  (DoubleRow, transposed out)
                ps = psv.tile([128, 512], F32, tag="vmm", name=f"psv{b}_{jt}")
                for itp in range(2):
                    nc.tensor.matmul(
                        ps[:], hn_sb[:, 2 * itp:2 * itp + 2, jt * 128:(jt + 1) * 128],
                        w_sb["wv"][:, 2 * itp:2 * itp + 2, :],
                        start=(itp == 0), stop=(itp == 1), perf_mode=DR)
                nc.vector.tensor_scalar(vT_sb[:, jt, :], ps[:], 1.0 / WS,
                                        None, Alu.mult)

            def s_tile(b, q_sb, k_sb, e_sb, jt):
                # e[:, jt, :] = exp(scale * k[:, :, jt-tile]^T @ q + EXPB)
                ps = psq.tile([128, N], F32, tag="mm", name=f"pss{b}_{jt}")
                for ctp in range(2):
                    lhs = k_sb[:, 2 * ctp:2 * ctp + 2, jt * 128:(jt + 1) * 128]
                    for ch in range(NCH):
                        cs = slice(ch * 512, (ch + 1) * 512)
                        nc.tensor.matmul(ps[:, cs], lhs,
                                         q_sb[:, 2 * ctp:2 * ctp + 2, cs],
                                         start=(ctp == 0), stop=(ctp == 1),
                                         perf_mode=DR)
                nc.scalar.activation(e_sb[:, jt, :], ps[:], Act.Exp,
                                     bias=expb_sb[:], scale=SCALE)

            def dsum_make(b):
                psd = [pssm.tile([1, 512], F32, tag="sm", name=f"d{b}_{ch}")
                       for ch in range(NCH)]
                return psd

            def dsum_acc(b, psd, e_sb, jts):
                # accumulate D rows for the given j-tiles (emitted one round
                # behind the S tiles so the exp evictions are always ready)
                for ch in range(NCH):
                    cs = slice(ch * 512, (ch + 1) * 512)
                    for jt in jts:
                        nc.tensor.matmul(psd[ch][:], ones_sb[:],
                                         e_sb[:, jt, cs],
                                         start=(jt == 0), stop=(jt == NT - 1))

            def dsum_tail(b, psd):
                drow = rp.tile([1, N], BF16, tag="drow", name=f"dr{b}")
                recip_sb = rp.tile([128, N], F32, tag="recip", name=f"rc{b}")
                for ch in range(NCH):
                    cs = slice(ch * 512, (ch + 1) * 512)
                    nc.scalar.copy(drow[:, cs], psd[ch][:])
                for ch in range(NCH):
                    cs = slice(ch * 512, (ch + 1) * 512)
                    bc = pssm.tile([128, 512], F32, tag="sm", name=f"bc{b}_{ch}")
                    nc.tensor.matmul(bc[:], onescol_sb[:], drow[:, cs],
                                     start=True, stop=True)
                    nc.vector.reciprocal_approx_fast(recip_sb[:, cs], bc[:])
                return recip_sb

            def pv_mms(b, vT_sb, e_sb, ct):
                ps = psq.tile([128, N], F32, tag="mm", name=f"pso{b}_{ct}")
                for jtp in range(4):
                    lhs = vT_sb[:, 2 * jtp:2 * jtp + 2, ct * 128:(ct + 1) * 128]
                    for ch in range(NCH):
                        cs = slice(ch * 512, (ch + 1) * 512)
                        nc.tensor.matmul(ps[:, cs], lhs,
                                         e_sb[:, 2 * jtp:2 * jtp + 2, cs],
                                         start=(jtp == 0), stop=(jtp == 3),
                                         perf_mode=DR)
                return ps

            out_engs = [nc.sync, nc.scalar, nc.gpsimd, nc.sync]

            def proj_tile(b, ou_sb, ot, evict):
                # evict: ('recip', recip_sb, xb_sb) -> o = ps*recip + xb
                #        ('act'|'dve', xts)        -> o = ps/(WS*OUS)+bpp, + x
                ps = psq.tile([128, N], F32, tag="mm", name=f"psp{b}_{ot}")
                for ctp in range(2):
                    lhs = w_sb["wp"][:, 2 * ctp:2 * ctp + 2, ot * 128:(ot + 1) * 128]
                    for ch in range(NCH):
                        cs = slice(ch * 512, (ch + 1) * 512)
                        nc.tensor.matmul(ps[:, cs], lhs,
                                         ou_sb[:, 2 * ctp:2 * ctp + 2, cs],
                                         start=(ctp == 0), stop=(ctp == 1),
                                         perf_mode=DR)
                o_sb = outp.tile([128, N], F32, tag="o", name=f"o{b}_{ot}")
                bias = vec_sb["bpp"][:, ot:ot + 1]
                if evict[0] == 'recip':
                    _, recip_sb, xb_sb = evict
                    nc.vector.tensor_tensor(o_sb[:], ps[:], recip_sb[:],
                                            Alu.mult)
                    nc.vector.tensor_tensor(o_sb[:], o_sb[:], xb_sb[:, ot, :],
                                            Alu.add)
                else:
                    kind, xts = evict
                    if kind == 'act':
                        nc.scalar.activation(o_sb[:], ps[:], Act.Identity,
                                             bias=bias, scale=1.0 / (WS * OUS))
                    else:
                        nc.vector.tensor_scalar(o_sb[:], ps[:],
                                                1.0 / (WS * OUS), bias,
                                                Alu.mult, Alu.add)
                    nc.vector.tensor_tensor(o_sb[:], o_sb[:], xts[ot][:],
                                            Alu.add)
                out_engs[ot].dma_start(out_ext[b, :, ot, :], o_sb[:])

            # ---- software pipeline over the two batch items ----
            # GroupNorm(0) with warmup bursts gated on its progress;
            # stats run in DMA-arrival order (t3 lands first on gpsimd).
            mv0 = smallp.tile([128, CT, 2], F32, tag="mv", name="mv0")
            for t in (0, 3, 2, 1):
                gn_stats(0, x0, mv0, t)
                warm_poke(mv0[:, t, :])
                warm_burst(6)
            h0 = gn_tail(0, x0, mv0, warm=True)

            # Q/K stretch for item 0 (ACT evictions) with item 1's GroupNorm
            # stats interleaved on the otherwise-idle DVE.
            q0 = qkp.tile([128, CT, N], F8, tag="q", name="q0")
            k0 = qkp.tile([128, CT, N], F8, tag="k", name="k0")
            mv1 = smallp.tile([128, CT, 2], F32, tag="mv", name="mv1")
            for t in range(CT):
                qk_tile(0, h0, k0, "wk", "bk", t, on_act=True)
                qk_tile(0, h0, q0, "wq", "bq", t, on_act=False)
                gn_stats(1, x1, mv1, t)
            h1 = gn_tail(1, x1, mv1)

            v0 = vp.tile([128, NT, 512], F8, tag="vT", name="vT0")
            for jt in range(NT):
                v_tile(0, h0, v0, jt)

            # item0 S-phase interleaved with item1 QKV + item0 D-sums (lagged)
            e0 = ep.tile([128, NT, N], F8, tag="e", name="e0")
            q1 = qkp.tile([128, CT, N], F8, tag="q", name="q1")
            k1 = qkp.tile([128, CT, N], F8, tag="k", name="k1")
            v1 = vp.tile([128, NT, 512], F8, tag="vT", name="vT1")
            psd0 = dsum_make(0)
            for r in range(CT):
                s_tile(0, q0, k0, e0, 2 * r)
                s_tile(0, q0, k0, e0, 2 * r + 1)
                qk_tile(1, h1, k1, "wk", "bk", r, on_act=True)
                qk_tile(1, h1, q1, "wq", "bq", r, on_act=False)
                v_tile(1, h1, v1, 2 * r)
                v_tile(1, h1, v1, 2 * r + 1)
                if r > 0:
                    dsum_acc(0, psd0, e0, (2 * r - 2, 2 * r - 1))
            dsum_acc(0, psd0, e0, (NT - 2, NT - 1))
            # pv0: PSUM evicted on ACT as a pure 1/256 cast (no recip
            # dependency); normalization happens at proj0's eviction.
            ou0 = oup.tile([128, CT, N], F8, tag="ou", name="ou0")
            for ct in range(CT):
                ps = pv_mms(0, v0, e0, ct)
                nc.scalar.mul(ou0[:, ct, :], ps[:], OCAST)
            r0 = dsum_tail(0, psd0)
            xb0 = xbp.tile([128, CT, N], BF16, tag="xb", name="xb0")
            for t in range(CT):
                nc.scalar.activation(xb0[:, t, :], x0[t][:], Act.Identity,
                                     bias=vec_sb["bpp"][:, t:t + 1])

            # item0 proj interleaved with item1 S-phase + item1 D-sums
            e1 = ep.tile([128, NT, N], F8, tag="e", name="e1")
            psd1 = dsum_make(1)
            for r in range(CT):
                proj_tile(0, ou0, r, ('recip', r0, xb0))
                s_tile(1, q1, k1, e1, 2 * r)
                s_tile(1, q1, k1, e1, 2 * r + 1)
                if r > 0:
                    dsum_acc(1, psd1, e1, (2 * r - 2, 2 * r - 1))
            # pv1-ct0's matmuls cover the wait for the last exp1 eviction
            ou1 = oup.tile([128, CT, N], F8, tag="ou", name="ou1")
            dsum_acc(1, psd1, e1, (NT - 2,))
            ps10 = pv_mms(1, v1, e1, 0)
            dsum_acc(1, psd1, e1, (NT - 1,))
            r1 = dsum_tail(1, psd1)
            nc.vector.tensor_tensor(ou1[:, 0, :], ps10[:], r1[:], Alu.mult)
            for ct in range(1, CT):
                ps = pv_mms(1, v1, e1, ct)
                nc.vector.tensor_tensor(ou1[:, ct, :], ps[:], r1[:], Alu.mult)
            for r in range(CT):
                proj_tile(1, ou1, r, ('act' if r % 2 == 0 else 'dve', x1))

    nc.compile()
    return nc


def _prep_vec(v):
    # [C] f32 -> [128, CT] with v_sb[p, t] = v[t*128 + p]
    return np.ascontiguousarray(
        np.asarray(v, dtype=np.float32).reshape(CT, 128).T)


def _prep_w(w):
    # [C, C] (out, in) -> lhsT layout [128, CT, 512] fp8e4, pre-scaled by WS:
    # w_sb[p, it, o] = w[o, it*128 + p] * WS
    wT = np.asarray(w, dtype=np.float32).T * WS
    arr = wT.reshape(CT, 128, C).transpose(1, 0, 2)
    return np.clip(np.ascontiguousarray(arr), -240.0, 240.0).astype(
        ml_dtypes.float8_e4m3)


def kernel(x, gamma, beta, wq, bq, wk, bk, wv, bv, wp, bp):
    from concourse.bass_utils import run_bass_kernel_spmd

    nc = _CACHE.get("nc")
    if nc is None:
        nc = _CACHE["nc"] = _build_bass()

    x = np.asarray(x, dtype=np.float32)
    # [16, C, H, W] -> [16, 128, CT, N] bf16
    xr = np.ascontiguousarray(
        x.reshape(B_TOT, CT, 128, N).transpose(0, 2, 1, 3)).astype(
        ml_dtypes.bfloat16)

    bpp = np.asarray(wp, np.float32) @ np.asarray(bv, np.float32) \
        + np.asarray(bp, np.float32)
    sel = np.kron(np.eye(128 // GS, dtype=np.float32),
                  np.full((GS, GS), 1.0 / GS, dtype=np.float32))
    cb = np.empty((128, CB_W), dtype=np.float32)
    for i, v in enumerate((gamma, beta, bq, bk, bpp)):
        cb[:, i * CT:(i + 1) * CT] = _prep_vec(v)
    cb[:, NVEC * CT:] = sel
    common = {
        "wq": _prep_w(wq), "wk": _prep_w(wk), "wv": _prep_w(wv),
        "wp": _prep_w(wp), "cb": cb,
    }
    in_maps = [
        {"x": np.ascontiguousarray(xr[c * BPC:(c + 1) * BPC]), **common}
        for c in range(NCORES)
    ]
    res = run_bass_kernel_spmd(nc, in_maps, core_ids=list(range(NCORES)))
    # [BPC, 128, CT, N] per core -> [16, C, H, W]
    out = np.concatenate([r["out"] for r in res.results], axis=0)
    return np.ascontiguousarray(
        out.transpose(0, 2, 1, 3)).reshape(B_TOT, C, H, W)
